# revision 4
# baseline (speedup 1.0000x reference)
"""MHA kernel for Trainium2, 8 NeuronCores.

Sharding: core c -> batch b = c//2, head-block hb = c%2 (8 of 16 heads).
Tensor-parallel within a batch: Wq/Wk/Wv column-sliced, Wo row-sliced;
each core emits a partial output [2048, 1024]; host sums the two partials
per batch and adds the bias (the "all-reduce" of row-parallel Wo done at
unshard time).

Per-core dataflow (all matmuls float32r = tf32-class, fp32 accumulate):
  xT   = PE-transpose(x)                     [dim,tok] 8x[128,2048]
  QT/KT[p] = (x @ W)^T via lhsT=W, rhs=xT    4x[128,2048] (pair p = 2 heads)
  V'   = x @ Wv with ones column per head    -> DRAM scratch [128,520]x16
  S^T duo: row-split K=64 pair (tile_position (0,0)/(64,0)) -> [128,1024] psum
  P^T  = exp(0.125 * S^T) on ACT             -> sbuf f32r
  O^T  = V'_h.T @ P^T_h (M=65, row 64 = softmax denominator)
  norm = recip(den) bcast via K=1 matmul, DVE multiply -> OT[p]
  out  = OT.T @ Wo (partial)                 [2048,1024] f32
"""
import numpy as np

import concourse.bacc as bacc
import concourse.mybir as mybir
from concourse.tile import TileContext
from concourse.bass_utils import run_bass_kernel_spmd

F32 = mybir.dt.float32
F32R = mybir.dt.float32r
AF = mybir.ActivationFunctionType

N = 2048      # tokens per batch
DIM = 1024    # model dim
HL = 512      # local inner (8 heads x 64)
NP = 4        # local head pairs
NJ = N // 128  # kv tiles
NQT = N // 512  # q tiles of 512
NK = DIM // 128  # contraction tiles

_CACHE = {}


def build():
    nc = bacc.Bacc(None, target_bir_lowering=False)
    x_d = nc.declare_dram_parameter("x", [N, DIM], F32R, isOutput=False)
    wq_d = nc.declare_dram_parameter("wq", [DIM, HL], F32R, isOutput=False)
    wk_d = nc.declare_dram_parameter("wk", [DIM, HL], F32R, isOutput=False)
    wv_d = nc.declare_dram_parameter("wv", [DIM, HL], F32R, isOutput=False)
    wo_d = nc.declare_dram_parameter("wo", [HL, DIM], F32R, isOutput=False)
    ones_d = nc.declare_dram_parameter("ones", [128, 64], F32R, isOutput=False)
    ident_d = nc.declare_dram_parameter("ident", [128, 128], F32R, isOutput=False)
    out_d = nc.declare_dram_parameter("out", [N, DIM], F32, isOutput=True)

    with TileContext(nc) as tc:
        with (
            tc.tile_pool(name="big", bufs=8) as big,      # xT then OT+Wo slots
            tc.tile_pool(name="qt", bufs=4) as qtp,
            tc.tile_pool(name="kt", bufs=4) as ktp,
            tc.tile_pool(name="w", bufs=9) as wp,
            tc.tile_pool(name="wsm", bufs=32) as wsm,
            tc.tile_pool(name="xin", bufs=2) as xinp,
            tc.tile_pool(name="pt", bufs=3) as ptp,
            tc.tile_pool(name="vp", bufs=18) as vpp,
            tc.tile_pool(name="st", bufs=3) as stp,
            tc.tile_pool(name="cn", bufs=1) as cn,
            tc.tile_pool(name="sps", bufs=2, space="PSUM") as spsp,
            tc.tile_pool(name="acc", bufs=1, space="PSUM") as accp,
            tc.tile_pool(name="dram", bufs=16, space="DRAM") as drp,
        ):
            ident = cn.tile([128, 128], F32R, name="ident")
            nc.sync.dma_start(out=ident[:], in_=ident_d[:])
            ones_sb = cn.tile([128, 64], F32R, name="ones_sb")
            nc.sync.dma_start(out=ones_sb[:], in_=ones_d[:])

            # ---- phase 1: transpose x -> xT[k] [128, N]
            xT = []
            for k in range(NK):
                t = big.tile([128, N], F32R, name=f"xT{k}", tag="big")
                xT.append(t)
            for tt in range(N // 128):
                xin = xinp.tile([128, DIM], F32R, name="xin")
                nc.sync.dma_start(out=xin[:], in_=x_d[tt * 128:(tt + 1) * 128, :])
                for k in range(NK):
                    tp = spsp.tile([128, 128], F32R, name="tp", tag="s")
                    nc.tensor.transpose(tp[:], xin[:, k * 128:(k + 1) * 128], ident[:])
                    nc.vector.tensor_copy(
                        out=xT[k][:, tt * 128:(tt + 1) * 128], in_=tp[:])

            # ---- phase 2+3 interleaved: V proj first, then per-pair
            # QT/KT projection + attention so proj PE hides under exp ACT.
            wv = []
            for k in range(NK):
                w = wp.tile([128, HL], F32R, name=f"wv{k}", tag="w")
                nc.sync.dma_start(out=w[:], in_=wv_d[k * 128:(k + 1) * 128, :])
                wv.append(w)
            vdr = []
            for tt in range(NJ):
                ps = spsp.tile([128, HL], F32, name="vps", tag="s")
                for k in range(NK):
                    nc.tensor.matmul(
                        ps[:], xT[k][:, tt * 128:(tt + 1) * 128], wv[k][:],
                        start=(k == 0), stop=(k == NK - 1))
                vst = ptp.tile([128, 520], F32R, name="vst", tag="pt")
                nc.vector.tensor_copy(out=vst[:, 64:520:65], in_=ones_sb[:, 0:8])
                for h in range(8):
                    nc.vector.tensor_copy(
                        out=vst[:, h * 65:h * 65 + 64],
                        in_=ps[:, h * 64:(h + 1) * 64])
                vd = drp.tile([128, 520], F32R, name=f"vd{tt}", tag="vd")
                nc.sync.dma_start(out=vd[:], in_=vst[:])
                vdr.append(vd)

            # Wo loads reuse freed xT slots
            wo = []
            for p in range(NP):
                w = big.tile([128, DIM], F32R, name=f"wo{p}", tag="big")
                nc.sync.dma_start(out=w[:], in_=wo_d[p * 128:(p + 1) * 128, :])
                wo.append(w)

            OT = [None] * NP
            for p in range(NP):
                # per-pair weight column slices [128, 128] x 8k for q and k
                wqp, wkp = [], []
                for k in range(NK):
                    w = wsm.tile([128, 128], F32R, name=f"wq{p}_{k}", tag="ws")
                    nc.sync.dma_start(
                        out=w[:], in_=wq_d[k * 128:(k + 1) * 128,
                                           p * 128:(p + 1) * 128])
                    wqp.append(w)
                    w = wsm.tile([128, 128], F32R, name=f"wk{p}_{k}", tag="ws")
                    nc.sync.dma_start(
                        out=w[:], in_=wk_d[k * 128:(k + 1) * 128,
                                           p * 128:(p + 1) * 128])
                    wkp.append(w)
                QTp = qtp.tile([128, N], F32R, name=f"QT{p}", tag="qt")
                KTp = ktp.tile([128, N], F32R, name=f"KT{p}", tag="kt")
                for t4 in range(NQT):
                    ps = spsp.tile([128, 512], F32, name="pps", tag="s")
                    for k in range(NK):
                        nc.tensor.matmul(
                            ps[:], wqp[k][:],
                            xT[k][:, t4 * 512:(t4 + 1) * 512],
                            start=(k == 0), stop=(k == NK - 1))
                    nc.vector.tensor_copy(
                        out=QTp[:, t4 * 512:(t4 + 1) * 512], in_=ps[:])
                    ps = spsp.tile([128, 512], F32, name="kps", tag="s")
                    for k in range(NK):
                        nc.tensor.matmul(
                            ps[:], wkp[k][:],
                            xT[k][:, t4 * 512:(t4 + 1) * 512],
                            start=(k == 0), stop=(k == NK - 1))
                    nc.vector.tensor_copy(
                        out=KTp[:, t4 * 512:(t4 + 1) * 512], in_=ps[:])

                pool_p = qtp if p % 2 == 0 else ktp
                OT[p] = pool_p.tile([128, N], F32R, name=f"OT{p}",
                                    tag="qt" if p % 2 == 0 else "kt")
                vtiles = []
                for j in range(NJ):
                    vj = vpp.tile([128, 130], F32R, name="vj", tag="vp")
                    nc.sync.dma_start(
                        out=vj[:], in_=vdr[j][:, p * 130:(p + 1) * 130])
                    vtiles.append(vj)
                for qt in range(NQT):
                    o_ps = [accp.tile([65, 512], F32, name=f"o{h}", tag=f"o{h}")
                            for h in range(2)]

                    def s_duo(j):
                        s_ps = spsp.tile([128, 1024], F32, name="s_ps", tag="s")
                        nc.tensor.matmul(
                            s_ps[:, 0:512], KTp[0:64, j * 128:(j + 1) * 128],
                            QTp[0:64, qt * 512:(qt + 1) * 512],
                            start=True, stop=True, tile_position=(0, 0))
                        nc.tensor.matmul(
                            s_ps[:, 512:1024], KTp[64:128, j * 128:(j + 1) * 128],
                            QTp[64:128, qt * 512:(qt + 1) * 512],
                            start=True, stop=True, tile_position=(64, 0))
                        return s_ps

                    # software pipeline: emit S(j+1) before attnV(j) so the
                    # in-order PE queue never stalls behind exp(j) on ACT.
                    s_cur = s_duo(0)
                    for j in range(NJ):
                        pt = ptp.tile([128, 1024], F32R, name="pt", tag="pt")
                        nc.scalar.activation(pt[:], s_cur[:], AF.Exp, scale=0.125)
                        if j + 1 < NJ:
                            s_cur = s_duo(j + 1)
                        for h in range(2):
                            nc.tensor.matmul(
                                o_ps[h][:], vtiles[j][:, h * 65:(h + 1) * 65],
                                pt[:, h * 512:(h + 1) * 512],
                                start=(j == 0), stop=(j == NJ - 1))
                    for h in range(2):
                        den = cn.tile([1, 512], F32R, name="den", tag="den",
                                      bufs=2)
                        with nc.allow_low_precision(reason="f32r==fp32 bits"):
                            nc.vector.reciprocal(den[0:1, :], o_ps[h][64:65, :])
                        bc_ps = accp.tile([64, 512], F32, name="bc", tag="bc",
                                          bufs=2)
                        nc.tensor.matmul(bc_ps[:], ones_sb[0:1, :], den[0:1, :],
                                         start=True, stop=True)
                        bc_sb = stp.tile([64, 512], F32, name="bc_sb", tag="st")
                        nc.vector.tensor_copy(out=bc_sb[:], in_=bc_ps[:])
                        nc.vector.tensor_tensor(
                            out=OT[p][h * 64:(h + 1) * 64,
                                      qt * 512:(qt + 1) * 512],
                            in0=o_ps[h][0:64, :], in1=bc_sb[:],
                            op=mybir.AluOpType.mult)

            # ---- phase 4: output projection (partial)
            for qs in range(N // 128):
                for dm in range(2):
                    ps = spsp.tile([128, 512], F32, name="ops", tag="s")
                    for p in range(NP):
                        nc.tensor.matmul(
                            ps[:], OT[p][:, qs * 128:(qs + 1) * 128],
                            wo[p][:, dm * 512:(dm + 1) * 512],
                            start=(p == 0), stop=(p == NP - 1))
                    ost = stp.tile([128, 512], F32, name="ost", tag="st")
                    nc.vector.tensor_copy(out=ost[:], in_=ps[:])
                    nc.sync.dma_start(
                        out=out_d[qs * 128:(qs + 1) * 128,
                                  dm * 512:(dm + 1) * 512],
                        in_=ost[:])
    nc.finalize()
    return nc


def kernel(x, Wq, Wk, Wv, Wo, bo, _trace=False):
    x = np.asarray(x, np.float32)
    Wq, Wk, Wv, Wo = (np.asarray(a, np.float32) for a in (Wq, Wk, Wv, Wo))
    bo = np.asarray(bo, np.float32)

    if "nc" not in _CACHE:
        _CACHE["nc"] = build()
    nc = _CACHE["nc"]

    ones_in = np.ones((128, 64), np.float32)
    ident_in = np.eye(128, dtype=np.float32)
    in_maps = []
    for c in range(8):
        b, hb = c // 2, c % 2
        sl = slice(hb * 512, (hb + 1) * 512)
        in_maps.append({
            "x": np.ascontiguousarray(x[b]),
            "wq": np.ascontiguousarray(Wq[:, sl]),
            "wk": np.ascontiguousarray(Wk[:, sl]),
            "wv": np.ascontiguousarray(Wv[:, sl]),
            "wo": np.ascontiguousarray(Wo[sl, :]),
            "ones": ones_in, "ident": ident_in,
        })
    res = run_bass_kernel_spmd(nc, in_maps, list(range(8)), trace=_trace)
    out = np.empty((4, N, DIM), np.float32)
    for b in range(4):
        out[b] = res.results[2 * b]["out"] + res.results[2 * b + 1]["out"] + bo
    if _trace:
        return out, res
    return out
